# revision 29
# baseline (speedup 1.0000x reference)
"""Trainium2 Bass kernel for nn_BaselineDistiller: grouped-expert MLP + MSE loss.

reference:
    h    = einsum('bne,neh->bnh', features, W1) + b1
    g    = gelu(h)                      # exact (erf) gelu
    pred = einsum('bnh,nhe->bne', g, W2) + b2
    out  = mean((pred - target)^2)

Strategy (8 NeuronCores, data-parallel over batch; measured ~161-164 us on HW):
  * Host: shard batch 8-ways; pre-transpose activations to expert-major
    [NE, E, B_shard] bf16 so the contraction dims (E, then H) land on SBUF
    partitions with no on-device transposes; fold b2 into the target.
  * Device (per core, per expert, per pair of 512-col batch tiles):
      mm1 (2 H-chunks x 2 tiles)           -> h.T in PSUM
      ACT gelu(+b1 per-partition bias), one op per chunk over the pair
        (free dim 1024, amortizing the ~310-cycle fixed ACT cost)
      mm2 accumulation group per tile: W2c0, W2c1, then (-I) @ target.T
        so PSUM ends up holding pred.T - target.T (subtract costs PE, not DVE)
      DVE bn_stats on the diff tile -> per-partition {count, mean, M2} pairs
    The loop is software-pipelined (next pair's mm1 emitted before current
    pair's mm2) so the in-order PE never idles waiting on gelu. Expert-0
    weights get dedicated tiles at the head of the DMA queue (first matmul
    ~12.5us after launch); the rest stream in 4-expert groups spread across
    expert slots so they never queue ahead of activations, and bn_stats
    results ship to DRAM per completed group so the kernel tail only waits
    on the last 16 tiles. The ScalarE (gelu, ~140 us with <2 us of gaps) is
    the saturated bottleneck engine; the remainder is NRT preamble, DMA
    ramp, and the fixed Tile exit barrier (~9.6 us).
  * Host: sum of squares = sum over tiles of M2s + n*mean^2, reduced in f64,
    divided by the element count.
"""

import contextlib
import ctypes
import json
import sys
import types

import ml_dtypes
import numpy as np

import concourse.bass as bass
import concourse.mybir as mybir
import concourse.tile as tile
from concourse import bass_utils
from concourse.bass import ts
from concourse.bass_utils import run_bass_kernel_spmd

B, NE, E, H = 16384, 32, 128, 256
C = 8              # cores
BS = B // C        # batch rows per core
BT = 512           # batch columns per matmul tile
NT = BS // BT
BF16 = mybir.dt.bfloat16
F32 = mybir.dt.float32

# ---------------------------------------------------------------------------
# Environment shims (idempotent):
#  1. antenv.axon_hooks — the image's antenv lacks it; provide the NTFF
#     profile hook via ctypes so trace=True works when a caller requests it.
#  2. upload_artifacts — no bucket access in this container; keep local.
#  3. This walrus build rejects instructions with >1 sync-wait; split the
#     extra waits onto NoOps at BIR-serialization time.
# ---------------------------------------------------------------------------
_AXON_SO = "/opt/axon/libaxon_pjrt.so"


def _make_ntff_hook(so_path):
    try:
        lib = ctypes.CDLL(so_path)
    except OSError:
        return None
    if not hasattr(lib, "axon_start_nrt_profile"):
        return None
    lib.axon_start_nrt_profile.argtypes = [ctypes.POINTER(ctypes.c_int64), ctypes.c_size_t]
    lib.axon_start_nrt_profile.restype = ctypes.c_int64
    lib.axon_stop_nrt_profile.argtypes = [ctypes.c_char_p]
    lib.axon_stop_nrt_profile.restype = ctypes.c_int64

    @contextlib.contextmanager
    def _hook(output_dir, device_ids):
        import jax

        jax.devices()
        if device_ids:
            ids = (ctypes.c_int64 * len(device_ids))(*device_ids)
            rc = lib.axon_start_nrt_profile(ids, len(device_ids))
        else:
            rc = lib.axon_start_nrt_profile(None, 0)
        if rc != 0:
            raise RuntimeError(f"axon_start_nrt_profile rc={rc}")
        try:
            yield
        finally:
            n = lib.axon_stop_nrt_profile(str(output_dir).encode())
            print(f"profile: {n} file(s) written to {output_dir}", file=sys.stderr)

    return _hook


if "antenv.axon_hooks" not in sys.modules:
    _mod = types.ModuleType("antenv.axon_hooks")
    _the_hook = _make_ntff_hook(_AXON_SO)
    _mod.get_axon_ntff_profile_hook = lambda: _the_hook
    sys.modules["antenv.axon_hooks"] = _mod

bass_utils.upload_artifacts = lambda tmpdir: str(tmpdir)

_MAXW = 1
if not getattr(bass.Bass, "_wait_split_installed", False):
    _orig_to_json_bytes = bass.Bass.to_json_bytes

    def _split_sync_waits(self, *a, **kw):
        bir = json.loads(_orig_to_json_bytes(self, *a, **kw))
        for fn in bir.get("functions", []):
            for blk in fn.get("blocks", []):
                new_insts = []
                for inst in blk.get("instructions", []):
                    si = inst.get("sync_info") or {}
                    waits = si.get("on_wait") or []
                    if len(waits) > _MAXW:
                        extra, keep = waits[:-_MAXW], waits[-_MAXW:]
                        for k in range(0, len(extra), _MAXW):
                            new_insts.append({
                                "debug": inst.get("debug", 0),
                                "engine": inst["engine"],
                                "ins": [], "outs": [],
                                "name": f"{inst['name']}_wsplit{k}",
                                "opcode": "NoOp",
                                "sync_info": {"on_update": [],
                                              "on_wait": extra[k:k + _MAXW]},
                            })
                        si["on_wait"] = keep
                    new_insts.append(inst)
                blk["instructions"] = new_insts
        return json.dumps(bir).encode()

    bass.Bass.to_json_bytes = _split_sync_waits
    bass.Bass._wait_split_installed = True


# ---------------------------------------------------------------------------
# Device kernel
# ---------------------------------------------------------------------------
NTILES = NE * NT          # batch tiles, per core
STATS_DIM = 6
NEGI_T = {0, 1, 2, 3}     # per-expert tile indices whose subtraction runs on PE


def _build_nc():
    nc = bass.Bass("TRN2", target_bir_lowering=False, debug=False)
    featd = nc.declare_dram_parameter("featT", [NE, E, BS], BF16, isOutput=False)
    targd = nc.declare_dram_parameter("targT", [NE, E, BS], BF16, isOutput=False)
    w1d = nc.declare_dram_parameter("w1", [E, NE, H], BF16, isOutput=False)
    w2d = nc.declare_dram_parameter("w2", [128, NE, 2, E], BF16, isOutput=False)
    b1d = nc.declare_dram_parameter("b1", [128, 2, NE], F32, isOutput=False)
    headd = nc.declare_dram_parameter("head", [128, 640], BF16, isOutput=False)
    statsd = nc.declare_dram_parameter("stats", [128, NTILES, STATS_DIM], F32,
                                       isOutput=True)
    has_sub = len(NEGI_T) < NT
    sqd = (nc.declare_dram_parameter("sq", [128, NTILES], F32, isOutput=True)
           if has_sub else None)

    with tile.TileContext(nc) as tc, contextlib.ExitStack() as ctx:
        wpool = ctx.enter_context(tc.tile_pool(name="weights", bufs=1))
        iopool = ctx.enter_context(tc.tile_pool(name="io", bufs=3))
        hpool = ctx.enter_context(tc.tile_pool(name="h", bufs=4))
        spool = ctx.enter_context(tc.tile_pool(name="scratch", bufs=2))
        stpool = ctx.enter_context(tc.tile_pool(name="stats", bufs=1))
        ph0p = ctx.enter_context(tc.tile_pool(name="ph0", bufs=1, space="PSUM"))
        ph1p = ctx.enter_context(tc.tile_pool(name="ph1", bufs=1, space="PSUM"))
        ppp = ctx.enter_context(tc.tile_pool(name="pp", bufs=2, space="PSUM"))

        # One packed head tile = [expert-0 W1 | -I | expert-0 W2] so a single
        # DMA (instead of four, each paying ~2us completion latency) unblocks
        # the first matmuls; the group tiles (which redundantly re-cover
        # expert 0) stream in behind.
        head_sb = wpool.tile([128, 640], BF16)
        nc.sync.dma_start(out=head_sb[:], in_=headd[:])
        negi_sb = head_sb[:, 256:384]
        b1_sb = wpool.tile([128, 2, NE], F32)
        nc.sync.dma_start(out=b1_sb[:], in_=b1d[:])
        GE = 4                      # experts per weight-DMA group
        NG = NE // GE
        w1g, w2g = [], []
        for g in range(NG):
            w1_sb = wpool.tile([E, GE, H], BF16, name=f"w1g{g}")
            w1g.append(w1_sb)
            w2_sb = wpool.tile([128, GE, 2, E], BF16, name=f"w2g{g}")
            w2g.append(w2_sb)

        stats_sb = stpool.tile([128, NTILES, STATS_DIM], F32)

        # Software-pipelined over pairs of 512-col batch tiles: emit the NEXT
        # pair's mm1 before the CURRENT pair's mm2 so the PE (in-order) can
        # fill its gelu-wait with independent work.
        pending = None   # (hact, targ_sb, n, t0, t1) awaiting mm2+bn_stats

        def flush(pending):
            hact, targ_sb, n, t0, t1 = pending
            pp0 = ppp.tile([128, BT], F32, name="pp0")
            pp1 = ppp.tile([128, BT], F32, name="pp1")
            for c in range(2):
                for pp_i, i in ((pp0, 0), (pp1, 1)):
                    nc.tensor.matmul(pp_i[:], lhsT=(head_sb[:, 384 + c * 128:512 + c * 128] if n == 0 else w2g[n // GE][:, n % GE, c, :]),
                                     rhs=hact[:, c, i, :],
                                     start=(c == 0), stop=False,
                                     skip_group_check=True)
            for pp_i, t in ((pp0, t0), (pp1, t1)):
                nc.tensor.matmul(pp_i[:], lhsT=negi_sb,
                                 rhs=targ_sb[:, ts(t, BT)],
                                 start=False, stop=True,
                                 skip_group_check=True)
            for pp_i, t in ((pp0, t0), (pp1, t1)):
                nc.vector.bn_stats(out=stats_sb[:, n * NT + t, :], in_=pp_i[:])

        for n in range(NE):
            feat_sb = iopool.tile([E, BS], BF16, tag="feat")
            nc.sync.dma_start(out=feat_sb[:], in_=featd[n])
            targ_sb = iopool.tile([E, BS], BF16, tag="targ")
            nc.sync.dma_start(out=targ_sb[:], in_=targd[n])
            if n == 0:
                nc.sync.dma_start(out=w1g[0][:], in_=w1d[:, 0:GE, :])
            elif n == 1:
                nc.sync.dma_start(out=w2g[0][:], in_=w2d[:, 0:GE, :, :])
            if n % GE == 1:
                g = n // GE + 1
                if g < NG:
                    nc.sync.dma_start(out=w1g[g][:], in_=w1d[:, ts(g, GE), :])
            if n % GE == 2:
                g = n // GE + 1
                if g < NG:
                    nc.sync.dma_start(out=w2g[g][:], in_=w2d[:, ts(g, GE), :, :])
            if n % GE == 2 and n > GE:
                # experts <= n-2 have flushed; ship the previous group's stats
                gd = n // GE - 1
                nc.sync.dma_start(out=statsd[:, ts(gd, GE * NT), :],
                                  in_=stats_sb[:, ts(gd, GE * NT), :])
            if n == NE - 1:
                gd = NG - 2
                nc.sync.dma_start(out=statsd[:, ts(gd, GE * NT), :],
                                  in_=stats_sb[:, ts(gd, GE * NT), :])
            for tp in range(NT // 2):
                t0, t1 = 2 * tp, 2 * tp + 1
                # mm1: h.T chunks; one weight load serves both tiles of a pair
                ph = [None, None]
                for c, pool_c in ((0, ph0p), (1, ph1p)):
                    ph[c] = pool_c.tile([128, 2, BT], F32, name=f"ph{c}")
                    for i, t in enumerate((t0, t1)):
                        nc.tensor.matmul(
                            ph[c][:, i, :],
                            lhsT=(head_sb[:, ts(c, 128)] if n == 0 else w1g[n // GE][:, n % GE, ts(c, 128)]),
                            rhs=feat_sb[:, ts(t, BT)],
                            start=True, stop=True,
                        )
                if pending is not None:
                    flush(pending)
                # gelu(+b1): one ACT op per chunk over both tiles (FD=1024)
                hact = hpool.tile([128, 2, 2, BT], BF16)   # [c, tile, BT]
                for c in range(2):
                    nc.scalar.activation(
                        hact[:, c, :, :], ph[c][:, :, :],
                        mybir.ActivationFunctionType.Gelu,
                        bias=b1_sb[:, c, n:n + 1], scale=1.0,
                    )
                pending = (hact, targ_sb, n, t0, t1)
        flush(pending)
        nc.sync.dma_start(out=statsd[:, ts(NG - 1, GE * NT), :],
                          in_=stats_sb[:, ts(NG - 1, GE * NT), :])
        if has_sub:
            nc.sync.dma_start(out=sqd[:], in_=sq_sb[:])
    return nc


LAST_RESULTS = None


def kernel(features, target_features, W1, b1, W2, b2):
    global LAST_RESULTS
    bf = ml_dtypes.bfloat16
    features = np.asarray(features)
    target_features = np.asarray(target_features)
    W1 = np.asarray(W1)
    b1 = np.asarray(b1)
    W2 = np.asarray(W2)
    b2 = np.asarray(b2)

    feat4 = features.reshape(C, BS, NE, E).transpose(0, 2, 3, 1).astype(bf)
    targ4 = (target_features - b2[None]).reshape(C, BS, NE, E).transpose(0, 2, 3, 1).astype(bf)
    w1h = W1.transpose(1, 0, 2).astype(bf)                          # [E, NE, H]
    w2h = W2.reshape(NE, 2, 128, E).transpose(2, 0, 1, 3).astype(bf)  # [128, NE, 2, E]
    b1h = np.ascontiguousarray(b1.reshape(NE, 2, 128).transpose(2, 1, 0).astype(np.float32))

    negi = (-np.eye(128)).astype(bf)
    head = np.ascontiguousarray(np.concatenate(
        [w1h[:, 0, :], negi, w2h[:, 0, 0, :], w2h[:, 0, 1, :]], axis=1))

    nc = _build_nc()
    in_maps = [
        {"featT": np.ascontiguousarray(feat4[c]),
         "targT": np.ascontiguousarray(targ4[c]),
         "w1": w1h, "w2": w2h, "b1": b1h, "head": head}
        for c in range(C)
    ]
    res = run_bass_kernel_spmd(nc, in_maps, list(range(C)))
    LAST_RESULTS = res
    # For NEGI_T tiles: stats[p, tile] = [n0, mean0, M2_0, n1, mean1, M2_1] of
    # the diff rows (bn_stats splits the 512 free elems into two 256-halves);
    # sum of squares = M2_0 + n0*mean0^2 + M2_1 + n1*mean1^2.
    # For the rest: sq[p, tile] = sum of squares directly.
    neg_mask = np.zeros(NTILES, dtype=bool)
    for n in range(NE):
        for j in NEGI_T:
            neg_mask[n * NT + j] = True
    total = 0.0
    for r in res.results:
        st = r["stats"].astype(np.float64)[:, neg_mask, :]
        total += (st[..., 2] + st[..., 0] * st[..., 1] ** 2
                  + st[..., 5] + st[..., 3] * st[..., 4] ** 2).sum()
        if "sq" in r:
            total += r["sq"].astype(np.float64)[:, ~neg_mask].sum()
    return np.array(total / (B * NE * E), dtype=np.float32)


# revision 30
# speedup vs baseline: 1.0113x; 1.0113x over previous
"""Trainium2 Bass kernel for nn_BaselineDistiller: grouped-expert MLP + MSE loss.

reference:
    h    = einsum('bne,neh->bnh', features, W1) + b1
    g    = gelu(h)                      # exact (erf) gelu
    pred = einsum('bnh,nhe->bne', g, W2) + b2
    out  = mean((pred - target)^2)

Strategy (8 NeuronCores, data-parallel over batch; measured ~161-164 us on HW):
  * Host: shard batch 8-ways; pre-transpose activations to expert-major
    [NE, E, B_shard] bf16 so the contraction dims (E, then H) land on SBUF
    partitions with no on-device transposes; fold b2 into the target.
  * Device (per core, per expert, per pair of 512-col batch tiles):
      mm1 (2 H-chunks x 2 tiles)           -> h.T in PSUM
      ACT gelu(+b1 per-partition bias), one op per chunk over the pair
        (free dim 1024, amortizing the ~310-cycle fixed ACT cost)
      mm2 accumulation group per tile: W2c0, W2c1, then (-I) @ target.T
        so PSUM ends up holding pred.T - target.T (subtract costs PE, not DVE)
      DVE bn_stats on the diff tile -> per-partition {count, mean, M2} pairs
    The loop is software-pipelined (next pair's mm1 emitted before current
    pair's mm2) so the in-order PE never idles waiting on gelu. Expert-0
    weights get dedicated tiles at the head of the DMA queue (first matmul
    ~12.5us after launch); the rest stream in 4-expert groups spread across
    expert slots so they never queue ahead of activations, and bn_stats
    results ship to DRAM per completed group so the kernel tail only waits
    on the last 16 tiles. The ScalarE (gelu, ~140 us with <2 us of gaps) is
    the saturated bottleneck engine; the remainder is NRT preamble, DMA
    ramp, and the fixed Tile exit barrier (~9.6 us).
  * Host: sum of squares = sum over tiles of M2s + n*mean^2, reduced in f64,
    divided by the element count.
"""

import contextlib
import ctypes
import json
import sys
import types

import ml_dtypes
import numpy as np

import concourse.bass as bass
import concourse.mybir as mybir
import concourse.tile as tile
from concourse import bass_utils
from concourse.bass import ts
from concourse.bass_utils import run_bass_kernel_spmd

B, NE, E, H = 16384, 32, 128, 256
C = 8              # cores
BS = B // C        # batch rows per core
BT = 512           # batch columns per matmul tile
NT = BS // BT
BF16 = mybir.dt.bfloat16
F32 = mybir.dt.float32

# ---------------------------------------------------------------------------
# Environment shims (idempotent):
#  1. antenv.axon_hooks — the image's antenv lacks it; provide the NTFF
#     profile hook via ctypes so trace=True works when a caller requests it.
#  2. upload_artifacts — no bucket access in this container; keep local.
#  3. This walrus build rejects instructions with >1 sync-wait; split the
#     extra waits onto NoOps at BIR-serialization time.
# ---------------------------------------------------------------------------
_AXON_SO = "/opt/axon/libaxon_pjrt.so"


def _make_ntff_hook(so_path):
    try:
        lib = ctypes.CDLL(so_path)
    except OSError:
        return None
    if not hasattr(lib, "axon_start_nrt_profile"):
        return None
    lib.axon_start_nrt_profile.argtypes = [ctypes.POINTER(ctypes.c_int64), ctypes.c_size_t]
    lib.axon_start_nrt_profile.restype = ctypes.c_int64
    lib.axon_stop_nrt_profile.argtypes = [ctypes.c_char_p]
    lib.axon_stop_nrt_profile.restype = ctypes.c_int64

    @contextlib.contextmanager
    def _hook(output_dir, device_ids):
        import jax

        jax.devices()
        if device_ids:
            ids = (ctypes.c_int64 * len(device_ids))(*device_ids)
            rc = lib.axon_start_nrt_profile(ids, len(device_ids))
        else:
            rc = lib.axon_start_nrt_profile(None, 0)
        if rc != 0:
            raise RuntimeError(f"axon_start_nrt_profile rc={rc}")
        try:
            yield
        finally:
            n = lib.axon_stop_nrt_profile(str(output_dir).encode())
            print(f"profile: {n} file(s) written to {output_dir}", file=sys.stderr)

    return _hook


if "antenv.axon_hooks" not in sys.modules:
    _mod = types.ModuleType("antenv.axon_hooks")
    _the_hook = _make_ntff_hook(_AXON_SO)
    _mod.get_axon_ntff_profile_hook = lambda: _the_hook
    sys.modules["antenv.axon_hooks"] = _mod

bass_utils.upload_artifacts = lambda tmpdir: str(tmpdir)

_MAXW = 1
if not getattr(bass.Bass, "_wait_split_installed", False):
    _orig_to_json_bytes = bass.Bass.to_json_bytes

    def _split_sync_waits(self, *a, **kw):
        bir = json.loads(_orig_to_json_bytes(self, *a, **kw))
        for fn in bir.get("functions", []):
            for blk in fn.get("blocks", []):
                new_insts = []
                for inst in blk.get("instructions", []):
                    si = inst.get("sync_info") or {}
                    waits = si.get("on_wait") or []
                    if len(waits) > _MAXW:
                        extra, keep = waits[:-_MAXW], waits[-_MAXW:]
                        for k in range(0, len(extra), _MAXW):
                            new_insts.append({
                                "debug": inst.get("debug", 0),
                                "engine": inst["engine"],
                                "ins": [], "outs": [],
                                "name": f"{inst['name']}_wsplit{k}",
                                "opcode": "NoOp",
                                "sync_info": {"on_update": [],
                                              "on_wait": extra[k:k + _MAXW]},
                            })
                        si["on_wait"] = keep
                    new_insts.append(inst)
                blk["instructions"] = new_insts
        return json.dumps(bir).encode()

    bass.Bass.to_json_bytes = _split_sync_waits
    bass.Bass._wait_split_installed = True


# ---------------------------------------------------------------------------
# Device kernel
# ---------------------------------------------------------------------------
NTILES = NE * NT          # batch tiles, per core
STATS_DIM = 6
NEGI_T = {0, 1, 2, 3}     # per-expert tile indices whose subtraction runs on PE


def _build_nc():
    nc = bass.Bass("TRN2", target_bir_lowering=False, debug=False)
    featd = nc.declare_dram_parameter("featT", [NE, E, BS], BF16, isOutput=False)
    targd = nc.declare_dram_parameter("targT", [NE, E, BS], BF16, isOutput=False)
    w1d = nc.declare_dram_parameter("w1", [E, NE, H], BF16, isOutput=False)
    w2d = nc.declare_dram_parameter("w2", [128, NE, 2, E], BF16, isOutput=False)
    headd = nc.declare_dram_parameter("head", [128, 768], BF16, isOutput=False)
    statsd = nc.declare_dram_parameter("stats", [128, NTILES, STATS_DIM], F32,
                                       isOutput=True)
    has_sub = len(NEGI_T) < NT
    sqd = (nc.declare_dram_parameter("sq", [128, NTILES], F32, isOutput=True)
           if has_sub else None)

    with tile.TileContext(nc) as tc, contextlib.ExitStack() as ctx:
        wpool = ctx.enter_context(tc.tile_pool(name="weights", bufs=1))
        iopool = ctx.enter_context(tc.tile_pool(name="io", bufs=3))
        hpool = ctx.enter_context(tc.tile_pool(name="h", bufs=4))
        spool = ctx.enter_context(tc.tile_pool(name="scratch", bufs=2))
        stpool = ctx.enter_context(tc.tile_pool(name="stats", bufs=1))
        ph0p = ctx.enter_context(tc.tile_pool(name="ph0", bufs=1, space="PSUM"))
        ph1p = ctx.enter_context(tc.tile_pool(name="ph1", bufs=1, space="PSUM"))
        ppp = ctx.enter_context(tc.tile_pool(name="pp", bufs=2, space="PSUM"))

        # One packed head tile = [expert-0 W1 | -I | expert-0 W2 | b1-as-bits]
        # so a single DMA (instead of five, each costing ~600ns of SP
        # descriptor time plus ~2us completion latency) unblocks the first
        # matmuls. It is issued right AFTER feat0 (the longest head transfer)
        # inside the loop; the group tiles (which redundantly re-cover expert
        # 0) stream in behind.
        head_sb = wpool.tile([128, 768], BF16)
        negi_sb = head_sb[:, 256:384]
        b1f = head_sb[:, 640:768].bitcast(F32)       # [128, 64] = b1[2, NE]
        GE = 4                      # experts per weight-DMA group
        NG = NE // GE
        w1g, w2g = [], []
        for g in range(NG):
            w1_sb = wpool.tile([E, GE, H], BF16, name=f"w1g{g}")
            w1g.append(w1_sb)
            w2_sb = wpool.tile([128, GE, 2, E], BF16, name=f"w2g{g}")
            w2g.append(w2_sb)

        stats_sb = stpool.tile([128, NTILES, STATS_DIM], F32)

        # Software-pipelined over pairs of 512-col batch tiles: emit the NEXT
        # pair's mm1 before the CURRENT pair's mm2 so the PE (in-order) can
        # fill its gelu-wait with independent work.
        pending = None   # (hact, targ_sb, n, t0, t1) awaiting mm2+bn_stats

        def flush(pending):
            hact, targ_sb, n, t0, t1 = pending
            pp0 = ppp.tile([128, BT], F32, name="pp0")
            pp1 = ppp.tile([128, BT], F32, name="pp1")
            for c in range(2):
                for pp_i, i in ((pp0, 0), (pp1, 1)):
                    nc.tensor.matmul(pp_i[:], lhsT=(head_sb[:, 384 + c * 128:512 + c * 128] if n == 0 else w2g[n // GE][:, n % GE, c, :]),
                                     rhs=hact[:, c, i, :],
                                     start=(c == 0), stop=False,
                                     skip_group_check=True)
            for pp_i, t in ((pp0, t0), (pp1, t1)):
                nc.tensor.matmul(pp_i[:], lhsT=negi_sb,
                                 rhs=targ_sb[:, ts(t, BT)],
                                 start=False, stop=True,
                                 skip_group_check=True)
            for pp_i, t in ((pp0, t0), (pp1, t1)):
                nc.vector.bn_stats(out=stats_sb[:, n * NT + t, :], in_=pp_i[:])

        for n in range(NE):
            feat_sb = iopool.tile([E, BS], BF16, tag="feat")
            nc.sync.dma_start(out=feat_sb[:], in_=featd[n])
            if n == 0:
                nc.sync.dma_start(out=head_sb[:], in_=headd[:])
            targ_sb = iopool.tile([E, BS], BF16, tag="targ")
            nc.sync.dma_start(out=targ_sb[:], in_=targd[n])
            if n == 0:
                nc.sync.dma_start(out=w1g[0][:], in_=w1d[:, 0:GE, :])
            elif n == 1:
                nc.sync.dma_start(out=w2g[0][:], in_=w2d[:, 0:GE, :, :])
            if n % GE == 1:
                g = n // GE + 1
                if g < NG:
                    nc.sync.dma_start(out=w1g[g][:], in_=w1d[:, ts(g, GE), :])
            if n % GE == 2:
                g = n // GE + 1
                if g < NG:
                    nc.sync.dma_start(out=w2g[g][:], in_=w2d[:, ts(g, GE), :, :])
            if n % GE == 2 and n > GE:
                # experts <= n-2 have flushed; ship the previous group's stats
                gd = n // GE - 1
                nc.sync.dma_start(out=statsd[:, ts(gd, GE * NT), :],
                                  in_=stats_sb[:, ts(gd, GE * NT), :])
            if n == NE - 1:
                gd = NG - 2
                nc.sync.dma_start(out=statsd[:, ts(gd, GE * NT), :],
                                  in_=stats_sb[:, ts(gd, GE * NT), :])
            for tp in range(NT // 2):
                t0, t1 = 2 * tp, 2 * tp + 1
                # mm1: h.T chunks; one weight load serves both tiles of a pair
                ph = [None, None]
                for c, pool_c in ((0, ph0p), (1, ph1p)):
                    ph[c] = pool_c.tile([128, 2, BT], F32, name=f"ph{c}")
                    for i, t in enumerate((t0, t1)):
                        nc.tensor.matmul(
                            ph[c][:, i, :],
                            lhsT=(head_sb[:, ts(c, 128)] if n == 0 else w1g[n // GE][:, n % GE, ts(c, 128)]),
                            rhs=feat_sb[:, ts(t, BT)],
                            start=True, stop=True,
                        )
                if pending is not None:
                    flush(pending)
                # gelu(+b1): one ACT op per chunk over both tiles (FD=1024)
                hact = hpool.tile([128, 2, 2, BT], BF16)   # [c, tile, BT]
                for c in range(2):
                    nc.scalar.activation(
                        hact[:, c, :, :], ph[c][:, :, :],
                        mybir.ActivationFunctionType.Gelu,
                        bias=b1f[:, c * NE + n:c * NE + n + 1], scale=1.0,
                    )
                pending = (hact, targ_sb, n, t0, t1)
        flush(pending)
        nc.sync.dma_start(out=statsd[:, ts(NG - 1, GE * NT), :],
                          in_=stats_sb[:, ts(NG - 1, GE * NT), :])
        if has_sub:
            nc.sync.dma_start(out=sqd[:], in_=sq_sb[:])
    return nc


LAST_RESULTS = None


def kernel(features, target_features, W1, b1, W2, b2):
    global LAST_RESULTS
    bf = ml_dtypes.bfloat16
    features = np.asarray(features)
    target_features = np.asarray(target_features)
    W1 = np.asarray(W1)
    b1 = np.asarray(b1)
    W2 = np.asarray(W2)
    b2 = np.asarray(b2)

    feat4 = features.reshape(C, BS, NE, E).transpose(0, 2, 3, 1).astype(bf)
    targ4 = (target_features - b2[None]).reshape(C, BS, NE, E).transpose(0, 2, 3, 1).astype(bf)
    w1h = W1.transpose(1, 0, 2).astype(bf)                          # [E, NE, H]
    w2h = W2.reshape(NE, 2, 128, E).transpose(2, 0, 1, 3).astype(bf)  # [128, NE, 2, E]
    b1h = np.ascontiguousarray(b1.reshape(NE, 2, 128).transpose(2, 1, 0).astype(np.float32))

    negi = (-np.eye(128)).astype(bf)
    head = np.ascontiguousarray(np.concatenate(
        [w1h[:, 0, :].view(np.uint16), negi.view(np.uint16),
         w2h[:, 0, 0, :].view(np.uint16), w2h[:, 0, 1, :].view(np.uint16),
         b1h.reshape(128, 64).view(np.uint16)],
        axis=1)).view(bf)

    nc = _build_nc()
    in_maps = [
        {"featT": np.ascontiguousarray(feat4[c]),
         "targT": np.ascontiguousarray(targ4[c]),
         "w1": w1h, "w2": w2h, "head": head}
        for c in range(C)
    ]
    res = run_bass_kernel_spmd(nc, in_maps, list(range(C)))
    LAST_RESULTS = res
    # For NEGI_T tiles: stats[p, tile] = [n0, mean0, M2_0, n1, mean1, M2_1] of
    # the diff rows (bn_stats splits the 512 free elems into two 256-halves);
    # sum of squares = M2_0 + n0*mean0^2 + M2_1 + n1*mean1^2.
    # For the rest: sq[p, tile] = sum of squares directly.
    neg_mask = np.zeros(NTILES, dtype=bool)
    for n in range(NE):
        for j in NEGI_T:
            neg_mask[n * NT + j] = True
    total = 0.0
    for r in res.results:
        st = r["stats"].astype(np.float64)[:, neg_mask, :]
        total += (st[..., 2] + st[..., 0] * st[..., 1] ** 2
                  + st[..., 5] + st[..., 3] * st[..., 4] ** 2).sum()
        if "sq" in r:
            total += r["sq"].astype(np.float64)[:, ~neg_mask].sum()
    return np.array(total / (B * NE * E), dtype=np.float32)
